# revision 12
# baseline (speedup 1.0000x reference)
"""CenterLoss kernel for 8 Trainium2 NeuronCores.

loss = mean(distmat * onehot(labels)) over a (B, C) distmat where
distmat[i, j] = ||x_i - c_j||^2.  The mask selects exactly one element
per row, so  loss = (1/(B*C)) * sum_i ||x_i - c_{labels[i]}||^2.

Strategy: data-parallel over batch.  Each of the 8 cores takes 512 rows
of x and gathers its 512 center rows with a SINGLE dma_gather (the
MoE-style SWDGE gather instruction): one 994ns ucode invocation for all
512 descriptors instead of 4 serialized DMA_INDIRECTs at ~1.1us each.
Vector then does one subtract + one fused square-reduce over the whole
[128, 512] block and the [128, 1] partial lands in DRAM.  Host sums the
partials in float64 and divides by B*C.

dma_gather contract (see bass_interp._exec_InstDMAGatherAnt):
  - idxs are int16, wrapped into 16 partitions: idx k lives at
    [k % 16, k // 16] of a [128, ceil(n/16)] tile (partitions 16..127
    must hold valid values; we zero them).
  - out[p, c, :] = centers[idx[c*128 + p], :], so x is loaded with the
    matching "(n p) d -> p n d" layout.

Raw Bass (no Block): the Block entry/exit semaphore machinery and
drain/barrier cost ~0.7us and are not needed — cross-engine deps are
taken with standalone wait_ge instructions, and a final Sync-engine
drain guarantees the output store lands before kernel completion.
"""

import sys

if "/opt/trn_rl_repo" not in sys.path:
    sys.path.insert(0, "/opt/trn_rl_repo")

import numpy as np

import concourse.bass as bass
from concourse import library_config, mybir

NCORES = 8
B = 4096
D = 128
C = 20000
P = 128
BS = B // NCORES          # 512 rows per core
N = BS // P               # 4 row-chunks (free-dim) per partition
IW = BS // 16             # 32 idx columns in the 16-partition wrap


def build_bass() -> bass.Bass:
    nc = bass.Bass(num_swdge_queues=1)
    x = nc.declare_dram_parameter("x", [BS, D], mybir.dt.float32, isOutput=False)
    idxw = nc.declare_dram_parameter("idxw", [P, IW], mybir.dt.int16, isOutput=False)
    centers = nc.declare_dram_parameter(
        "centers", [C, D], mybir.dt.float32, isOutput=False
    )
    out = nc.declare_dram_parameter("out", [P, 1], mybir.dt.float32, isOutput=True)

    with (
        nc.sbuf_tensor([P, IW], mybir.dt.int16) as idx_t,
        nc.sbuf_tensor([P, N, D], mybir.dt.float32) as x_t,
        nc.sbuf_tensor([P, N, D], mybir.dt.float32) as g_t,
        nc.sbuf_tensor([P, N, D], mybir.dt.float32) as d_t,
        nc.sbuf_tensor([P, N, D], mybir.dt.float32) as sq_t,
        nc.sbuf_tensor([P, 1], mybir.dt.float32) as red_t,
        nc.semaphore("idx_sem") as idx_sem,
        nc.semaphore("x_sem") as x_sem,
        nc.semaphore("g_sem") as g_sem,
        nc.semaphore("v_sem") as v_sem,
        nc.semaphore("done_sem") as done_sem,
    ):
        # Input loads, issued on the Sync engine's HWDGE queue.  idxw
        # arrives with the 16-partition wrap in partitions 0..15 and
        # zeros in 16..127 (the gather ucode reads the full tile).
        nc.sync.dma_start(out=idx_t[:], in_=idxw[:]).then_inc(idx_sem, 16)
        nc.sync.dma_start(
            out=x_t[:], in_=x[:].rearrange("(n p) d -> p n d", p=P)
        ).then_inc(x_sem, 16)

        # gpsimd: switch to the ucode library holding DMAGatherAnt and
        # pre-load the valid-count register while the idx DMA is in
        # flight, then fire the single gather for all 512 rows.
        nc.gpsimd.load_library(library_config.mlp)
        nidx_reg = nc.gpsimd.to_reg(BS)
        nc.gpsimd.wait_ge(idx_sem, 16)
        nc.gpsimd.dma_gather(
            out_ap=g_t[:],
            in_ap=centers[:],
            idxs_ap=idx_t[:],
            num_idxs=BS,
            num_idxs_reg=nidx_reg,
            elem_size=D,
        ).then_inc(g_sem, 16)

        # vector: d = x - g, then accum = sum(d*d) per partition.
        nc.vector.wait_ge(x_sem, 16)
        nc.vector.wait_ge(g_sem, 16)
        nc.vector.tensor_tensor(
            out=d_t[:],
            in0=x_t[:],
            in1=g_t[:],
            op=mybir.AluOpType.subtract,
        ).then_inc(v_sem, 1)
        nc.vector.wait_ge(v_sem, 1)
        nc.vector.scalar_tensor_tensor(
            out=sq_t[:],
            in0=d_t[:],
            scalar=0.0,
            in1=d_t[:],
            op0=mybir.AluOpType.add,
            op1=mybir.AluOpType.mult,
            accum_out=red_t[:],
        ).then_inc(v_sem, 1)

        # Sync: store the [128, 1] partial and drain so the store lands
        # before the NEFF postamble runs.
        nc.sync.wait_ge(v_sem, 2)
        nc.sync.dma_start(out=out[:], in_=red_t[:]).then_inc(done_sem, 16)
        nc.sync.drain()

    # Encode ISA-subclass pseudo instructions (the gpsimd library reload)
    # into raw instruction words — the walrus custom-kernel path passes
    # InstISA bytes through verbatim and rejects unencoded pseudos.
    mybir.codegen_inst_isa_subclasses(nc)
    if not nc.is_finalized():
        nc.finalize()
    return nc


_NC = None


def _get_nc() -> bass.Bass:
    global _NC
    if _NC is None:
        _NC = build_bass()
    return _NC


def make_in_maps(x, labels, centers):
    x = np.ascontiguousarray(np.asarray(x, dtype=np.float32))
    labels = np.asarray(labels).astype(np.int16)
    centers = np.ascontiguousarray(np.asarray(centers, dtype=np.float32))
    in_maps = []
    for c in range(NCORES):
        sl = slice(c * BS, (c + 1) * BS)
        in_maps.append(
            {
                "x": np.ascontiguousarray(x[sl]),
                # idx k of this core at [k % 16, k // 16], and the whole
                # 16-partition wrap replicated to all 8 partition groups:
                # the gather ucode's tx/rx Q7 cpus each read their own
                # 16-partition copy of the tile.
                "idxw": np.ascontiguousarray(
                    np.tile(labels[sl].reshape(IW, 16).T, (P // 16, 1))
                ),
                "centers": centers,
            }
        )
    return in_maps


def reduce_outputs(results) -> np.ndarray:
    total = 0.0
    for r in results:
        total += float(np.sum(r["out"].astype(np.float64)))
    return np.array(np.float32(total / (B * C)))


def kernel(x, labels, centers) -> np.ndarray:
    from concourse.bass_utils import run_bass_kernel_spmd

    nc = _get_nc()
    in_maps = make_in_maps(x, labels, centers)
    res = run_bass_kernel_spmd(nc, in_maps, list(range(NCORES)))
    return reduce_outputs(res.results)
